# revision 19
# baseline (speedup 1.0000x reference)
"""Trainium2 Bass kernel for nn_GATTrafficPredictionModel.

Mathematical collapse exploited (holds for every input by construction of the
model, not by luck of the data):
  - h = broadcast(x[:, -1, :]) makes all N=512 node features identical per
    sample, and the adjacency is dense all-ones.
  - GAT attention scores e[i,j] = leakyrelu(s_src[i] + s_dst[j]) are therefore
    constant over (i, j), so softmax over neighbors is exactly uniform (1/512,
    exact in fp32), and the attention-weighted sum of identical rows
    reproduces the row itself.  Both GAT layers collapse to per-sample linear
    maps; a1/a2 attention vectors drop out entirely.

Collapsed computation (B=32, F=128, K=8, H=64, C=64, N=512):
    z      = x[:, -1, :]                          (B, F)
    u      = elu(z @ W_heads)  flattened heads    (B, K*H)
    w_row  = u @ W_out                            (B, C)
    S      = sum_n Wf.reshape(C, N, C)[:, n, :]   (C, C)
    out    = w_row @ S.T + bf                     (B, C)

Sharding: each of the 8 cores owns 8 output channels c' (8 contiguous rows
of Wf), reduces them to S^T[:, c'_range] on-device, and computes its disjoint
slice out^T[c'_range, :].  The tiny upstream GEMMs are replicated per core.

Optimizations over the previous (2127 ns) version:
  - Wf ships as fp8e3 (e3m4) instead of fp16 -- 256 KiB/core instead of 512.
    The quantizer uses error diffusion along n (the axis the device sums
    over): the residual of each cast is carried into the next element, so the
    *sum* of the shipped fp8 values matches the fp32 sum to within one
    quantum.  End-to-end rel err ~6e-4 (vs 1.1e-2 for naive fp8).
  - The n-reduction moves from DVE tensor_reduce (1x mode, ~2.2 us for 2048
    cols) to TensorE: matmuls against a constant block-identity mask
    contract 128 partition rows at a time at 2.4 GHz.  Only a short
    [*, 8*NI] -> [*, 8] tail reduce stays on DVE.
  - Optional 2-way column tiling (KV_MODE=coltile, NOT default): two concurrent
    matmul streams on array column groups 0-63 / 64-127 halve the PE
    streaming time for the Wf reduction.  The two partition-halves of S^T
    are summed for free inside the final matmul by duplicating w_row^T rows
    via a 0-stride lhs access pattern.
  - elu recombined as (relu(x) - 1) + exp(-relu(-x)) with one fused DVE
    scalar_tensor_tensor; final out = (o_p * sWf) + bf is one fused DVE
    tensor_scalar with both scalars riding as per-partition data (so the
    compiled program has no input-derived immediates).

Per-core DMA: wf 256 KiB fp8 + small pack ~201 KiB fp16 = ~457 KiB.
Shipping config (measured best, HW differential ~1.0-1.1 us/iter vs 2.1 us
baseline): MODE=plain FD=128, ELU=dve (exp on ACT, everything else on DVE),
FINAL=dve, separate small/wf DMAs.  Rejected by measurement: coltile (x2
slower -- tile_position forces a serialized LDWEIGHTS before every matmul),
FD=256 (+370 ns), fused single DMA (x4 slower -- kills cross-rep
pipelining), ELU=act (ACT's 3x400 ns serial chain binds).
"""

import os
import numpy as np
import ml_dtypes

import concourse.bass as bass
import concourse.bacc as bacc
import concourse.mybir as mybir
import concourse.tile as tile
from concourse.bass_utils import run_bass_kernel_spmd

N_CORES = 8
B, S_SEQ, F = 32, 12, 128
K, H, C, N = 8, 64, 64, 512
ROWS = C // N_CORES          # output channels per core
F32 = mybir.dt.float32
F16 = mybir.dt.float16
F8E3 = mybir.dt.float8e3
AF = mybir.ActivationFunctionType
ALU = mybir.AluOpType

MODE = os.environ.get("KV_MODE", "plain")     # coltile | plain
FD = int(os.environ.get("KV_FD", "128"))      # rhs cols per wf matmul (plain)
ELU = os.environ.get("KV_ELU", "dve")         # act | mid | dve
FINAL = os.environ.get("KV_FINAL", "dve")     # act | dve
FUSE = os.environ.get("KV_FUSE_DMA", "0") == "1"   # single input DMA per rep
if MODE == "coltile":
    NI = 8
    NKCHUNK = 16
else:
    NI = FD // 8                              # psum ni-width per c'
    NKCHUNK = 2048 // FD                      # accumulation steps per stream

# small-pack column layout (fp16): zt | wht | wot | aux-bits
ZT0 = 0
WHT0 = ZT0 + B
WOT0 = WHT0 + K * H
AUXBITS0 = WOT0 + 4 * C      # even => fp32 bitcast view is 4B aligned
BFT_COL = 0                  # aux fp32 col 0: bias (rows 0..ROWS)
SWF_COL = 1                  # aux fp32 col 1: Wf scale (rows 0..ROWS)
AUX_COLS = 2
SMALL_COLS = AUXBITS0 + 2 * AUX_COLS


def _emit_consts(nc, cpool, t):
    """Hoisted once per program: the block-identity reduction mask."""
    mask_s = cpool.tile([128, C], F8E3, tag="mask", name="mask")
    nc.sync.dma_start(mask_s[:], t["cst"][:])
    return mask_s


def _emit_body(nc, pool, wfpool, psum, t, mask_s, tc=None):
    """One full per-core computation; `t` maps dram tensor names to handles."""
    if FUSE:
        all_s = wfpool.tile([128, 2048 + 2 * SMALL_COLS], F8E3, tag="all")
        nc.sync.dma_start(all_s[:], t["wfall"][:])
        wf_view = all_s[:, 0:2048]
        wf_slices = [wf_view[:, 1024 * ci:1024 * (ci + 1)] for ci in range(2)]
        small_s = all_s[:, 2048:2048 + 2 * SMALL_COLS].bitcast(F16)
    else:
        small_t = pool.tile([128, SMALL_COLS], F16, tag="small")
        nc.sync.dma_start(small_t[:], t["small"][:])
        small_s = small_t[:]
        wf_tiles = [wfpool.tile([128, 1024], F8E3, tag=f"wfchunk{ci}",
                                name=f"wfchunk{ci}")
                    for ci in range(2)]
        for ci in range(2):
            nc.sync.dma_start(wf_tiles[ci][:],
                              t["wf"][:, 1024 * ci:1024 * (ci + 1)])
        wf_slices = [wf_tiles[ci][:] for ci in range(2)]

    zt_s = small_s[:, ZT0:ZT0 + B]
    wht_s = small_s[:, WHT0:WHT0 + K * H]
    wot_s = small_s[:, WOT0:WOT0 + 4 * C]
    aux_v = small_s[:, AUXBITS0:AUXBITS0 + 2 * AUX_COLS].bitcast(F32)
    bft_s = aux_v[0:ROWS, BFT_COL:BFT_COL + 1]
    swf_s = aux_v[0:ROWS, SWF_COL:SWF_COL + 1]

    # ---- u-pre = W_heads^T z  (4 chunks of 128 kh each) ---------------
    wh_p = psum.tile([128, 4 * B], F32, tag="whp")
    for j in range(4):
        nc.tensor.matmul(
            wh_p[:, B * j:B * (j + 1)],
            wht_s[:, 128 * j:128 * (j + 1)],
            zt_s,
            start=True, stop=True,
        )

    # ---- S^T from the Wf shard: PE mask-matmul reduction --------------
    # wf col layout (plain):   j = k*128 + c'l*16 + ni   (h = k*16 + ni)
    # wf col layout (coltile): j = k*128 + T*64 + c'l*8 + ni
    #                          (h = T*128 + k*8 + ni), T = array col group
    if MODE == "coltile":
        st_p = psum.tile([128, C], F32, tag="stp")
        for k in range(NKCHUNK):
            ci, off = k // 8, (k % 8) * 128
            for T in range(2):
                nc.tensor.matmul(
                    st_p[64 * T:64 * (T + 1), :],
                    mask_s[:],
                    wf_slices[ci][:, off + 64 * T:off + 64 * (T + 1)],
                    start=(k == 0), stop=(k == NKCHUNK - 1),
                )
        st_rows = 128
    else:
        kperchunk = NKCHUNK // 2
        st_p = psum.tile([C, 8 * NI], F32, tag="stp")
        for k in range(NKCHUNK):
            ci, off = k // kperchunk, (k % kperchunk) * FD
            nc.tensor.matmul(
                st_p[:],
                mask_s[:],
                wf_slices[ci][:, off:off + FD],
                start=(k == 0), stop=(k == NKCHUNK - 1),
            )
        st_rows = C

    # ---- elu: u = (relu(x) - 1) + exp(min(x, 0)) ----------------------
    u_s = pool.tile([128, 4 * B], F16, tag="u")
    e1_s = pool.tile([128, 4 * B], F16, tag="e1")
    if ELU == "dve":
        m_s = pool.tile([128, 4 * B], F16, tag="m")
        nc.vector.tensor_scalar_min(m_s[:], wh_p[:], 0.0)
        nc.scalar.activation(e1_s[:], m_s[:], AF.Exp)
        r1_s = pool.tile([128, 4 * B], F16, tag="r1")
        nc.vector.tensor_scalar(
            r1_s[:], wh_p[:], 0.0, -1.0, op0=ALU.max, op1=ALU.add)
        nc.vector.tensor_add(u_s[:], r1_s[:], e1_s[:])
    elif ELU == "mid":
        rneg_s = pool.tile([128, 4 * B], F16, tag="rneg")
        nc.scalar.activation(rneg_s[:], wh_p[:], AF.Relu, scale=-1.0)
        nc.scalar.activation(e1_s[:], rneg_s[:], AF.Exp, scale=-1.0)
        r1_s = pool.tile([128, 4 * B], F16, tag="r1")
        nc.vector.tensor_scalar(
            r1_s[:], wh_p[:], 0.0, -1.0, op0=ALU.max, op1=ALU.add)
        nc.vector.tensor_add(u_s[:], r1_s[:], e1_s[:])
    else:
        rneg_s = pool.tile([128, 4 * B], F16, tag="rneg")
        nc.scalar.activation(rneg_s[:], wh_p[:], AF.Relu, scale=-1.0)
        nc.scalar.activation(e1_s[:], rneg_s[:], AF.Exp, scale=-1.0)
        r_s = pool.tile([128, 4 * B], F16, tag="r")
        nc.scalar.activation(r_s[:], wh_p[:], AF.Relu)
        nc.vector.scalar_tensor_tensor(
            u_s[:], r_s[:], -1.0, e1_s[:], op0=ALU.add, op1=ALU.add)

    # ---- w_row^T = W_out^T u ------------------------------------------
    # coltile: also materialize a copy of w_row^T on partitions 64..127
    # (array col groups 2-3, concurrent with groups 0-1) so the final
    # matmul's 128-partition contraction sums the two S^T halves for free.
    wr_p = psum.tile([st_rows, B], F32, tag="wrp")
    halves = 2 if MODE == "coltile" else 1
    for j in range(4):
        wot_j = wot_s[:, C * j:C * (j + 1)]
        for hf in range(halves):
            nc.tensor.matmul(
                wr_p[64 * hf:64 * hf + 64, :] if halves == 2 else wr_p[:],
                wot_j, u_s[:, B * j:B * (j + 1)],
                start=(j == 0), stop=(j == 3),
            )
    wr_s = pool.tile([st_rows, B], F32, tag="wrs")
    nc.vector.tensor_copy(wr_s[:], wr_p[:])

    # ---- tail reduce over ni: S^T slice -------------------------------
    st_s = pool.tile([st_rows, ROWS], F32, tag="sts")
    nc.vector.tensor_reduce(
        st_s[:],
        st_p[:].rearrange("p (c n) -> p c n", n=NI),
        axis=mybir.AxisListType.X,
        op=ALU.add,
    )

    # ---- out^T[c' slice] = sWf * (S^T.T w_row^T) + bf -----------------
    o_p = psum.tile([ROWS, B], F32, tag="op")
    nc.tensor.matmul(o_p[:], st_s[:], wr_s[:], start=True, stop=True)
    o_s = pool.tile([ROWS, B], F32, tag="os")
    if FINAL == "act":
        nc.scalar.activation(o_s[:], o_p[:], AF.Identity,
                             bias=bft_s, scale=swf_s)
    else:
        nc.vector.tensor_scalar(
            o_s[:], o_p[:], swf_s, bft_s, op0=ALU.mult, op1=ALU.add)
    nc.sync.dma_start(t["out"][:], o_s[:])


def _build_nc(reps=1, loop_iters=None):
    nc = bacc.Bacc("TRN2", target_bir_lowering=False, debug=False,
                   num_devices=N_CORES)

    if FUSE:
        t = {
            "wfall": nc.dram_tensor("wfall", [128, 2048 + 2 * SMALL_COLS],
                                    F8E3, kind="ExternalInput"),
            "cst": nc.dram_tensor("cst", [128, C], F8E3, kind="ExternalInput"),
            "out": nc.dram_tensor("out", [ROWS, B], F32, kind="ExternalOutput"),
        }
    else:
        t = {
            "wf": nc.dram_tensor("wf", [128, 2048], F8E3, kind="ExternalInput"),
            "small": nc.dram_tensor("small", [128, SMALL_COLS], F16,
                                    kind="ExternalInput"),
            "cst": nc.dram_tensor("cst", [128, C], F8E3, kind="ExternalInput"),
            "out": nc.dram_tensor("out", [ROWS, B], F32, kind="ExternalOutput"),
        }

    with tile.TileContext(nc) as tc:
        with (
            tc.tile_pool(name="cpool", bufs=1) as cpool,
            tc.tile_pool(name="pool", bufs=int(os.environ.get("KV_POOL_BUFS", "3"))) as pool,
            tc.tile_pool(name="wfpool", bufs=int(os.environ.get("KV_WF_BUFS", "2"))) as wfpool,
            tc.tile_pool(name="psum", bufs=2, space=bass.MemorySpace.PSUM) as psum,
        ):
            mask_s = _emit_consts(nc, cpool, t)
            if loop_iters:
                tc.For_i_unrolled(
                    0, loop_iters, 1,
                    lambda iv: _emit_body(nc, pool, wfpool, psum, t, mask_s, tc),
                    max_unroll=int(os.environ.get("KV_UNROLL", "64")),
                )
            else:
                for _rep in range(reps):
                    _emit_body(nc, pool, wfpool, psum, t, mask_s, tc)

    nc.compile()
    return nc


_NC_CACHE = None
_last_in_maps = None


def _quant_wf_feedback(Wf):
    """fp8e3 quantization of Wf with error diffusion along n (the summed
    axis): sum_n q[:, n, :] == sum_n Wf[:, n, :] to within one quantum."""
    m = float(np.abs(Wf).max())
    swf = float(2.0 ** np.ceil(np.log2(m / 7.75))) if m > 0 else 1.0
    W = (Wf / swf).reshape(C, N, C).astype(np.float32)
    q = np.empty((C, N, C), dtype=ml_dtypes.float8_e3m4)
    carry = np.zeros((C, C), np.float32)
    for n in range(N):
        tgt = W[:, n, :] + carry
        qn = tgt.astype(ml_dtypes.float8_e3m4)
        carry = tgt - qn.astype(np.float32)
        q[:, n, :] = qn
    return q.reshape(C, N * C), swf


def _make_in_maps(x, W_heads, W_out, Wf, bf):
    x = np.ascontiguousarray(np.asarray(x, np.float32))
    W_heads = np.ascontiguousarray(np.asarray(W_heads, np.float32))
    W_out = np.ascontiguousarray(np.asarray(W_out, np.float32))
    Wf = np.ascontiguousarray(np.asarray(Wf, np.float32))
    bf = np.ascontiguousarray(np.asarray(bf, np.float32))

    small = np.zeros((128, SMALL_COLS), np.float16)
    small[:, ZT0:ZT0 + B] = x[:, -1, :].T                          # (128, 32)
    small[:, WHT0:WHT0 + K * H] = \
        W_heads.transpose(1, 0, 2).reshape(F, K * H)               # (128, 512)
    small[:, WOT0:WOT0 + 4 * C] = \
        W_out.reshape(4, 128, C).transpose(1, 0, 2).reshape(128, 4 * C)

    qWf, swf = _quant_wf_feedback(Wf)                              # (64, 32768)

    # constant block-identity mask: mask[p, c] = (p % 64 == c)
    maskh = np.zeros((128, C), dtype=ml_dtypes.float8_e3m4)
    pp = np.arange(128)
    maskh[pp, pp % C] = ml_dtypes.float8_e3m4(1.0)

    in_maps = []
    for core in range(N_CORES):
        shard = qWf[ROWS * core:ROWS * (core + 1)]                 # (8, 32768)
        sh = shard.reshape(ROWS, 256, 128)                         # [c'l, h, p]
        if MODE == "coltile":
            # h = T*128 + k*8 + ni ; col j = k*128 + T*64 + c'l*8 + ni
            g = sh.reshape(ROWS, 2, 16, 8, 128)                    # [c'l,T,k,ni,p]
            wf_host = np.ascontiguousarray(
                g.transpose(4, 2, 1, 0, 3)).reshape(128, 2048)     # [p,k,T,c'l,ni]
        else:
            # h = k*NI + ni ; col j = k*FD + c'l*NI + ni
            g = sh.reshape(ROWS, NKCHUNK, NI, 128)                 # [c'l,k,ni,p]
            wf_host = np.ascontiguousarray(
                g.transpose(3, 1, 0, 2)).reshape(128, 2048)        # [p,k,c'l,ni]

        aux = np.zeros((128, AUX_COLS), np.float32)
        aux[0:ROWS, BFT_COL] = bf[ROWS * core:ROWS * (core + 1)]
        aux[0:ROWS, SWF_COL] = swf
        small_c = small.copy()
        small_c[:, AUXBITS0:AUXBITS0 + 2 * AUX_COLS] = aux.view(np.float16)
        if FUSE:
            wfall = np.concatenate(
                [wf_host.view(np.uint8), small_c.view(np.uint8)], axis=1
            ).view(ml_dtypes.float8_e3m4)
            in_maps.append({"wfall": wfall, "cst": maskh})
        else:
            in_maps.append({"wf": wf_host, "small": small_c, "cst": maskh})
    return in_maps


def kernel(x, W_heads, a1_heads, a2_heads, W_out, a1_out, a2_out, Wf, bf):
    global _NC_CACHE
    if _NC_CACHE is None:
        _NC_CACHE = _build_nc()
    nc = _NC_CACHE

    in_maps = _make_in_maps(x, W_heads, W_out, Wf, bf)
    global _last_in_maps
    _last_in_maps = in_maps
    res = run_bass_kernel_spmd(nc, in_maps, list(range(N_CORES)))
    outT = np.concatenate([res.results[i]["out"] for i in range(N_CORES)], axis=0)
    return np.ascontiguousarray(outT.T)                            # (32, 64)


# revision 21
# speedup vs baseline: 1.6215x; 1.6215x over previous
"""Trainium2 Bass kernel for nn_GATTrafficPredictionModel.

Mathematical collapse exploited (holds for every input by construction of the
model, not by luck of the data):
  - h = broadcast(x[:, -1, :]) makes all N=512 node features identical per
    sample, and the adjacency is dense all-ones.
  - GAT attention scores e[i,j] = leakyrelu(s_src[i] + s_dst[j]) are therefore
    constant over (i, j), so softmax over neighbors is exactly uniform (1/512,
    exact in fp32), and the attention-weighted sum of identical rows
    reproduces the row itself.  Both GAT layers collapse to per-sample linear
    maps; a1/a2 attention vectors drop out entirely.

Collapsed computation (B=32, F=128, K=8, H=64, C=64, N=512):
    z      = x[:, -1, :]                          (B, F)
    u      = elu(z @ W_heads)  flattened heads    (B, K*H)
    w_row  = u @ W_out                            (B, C)
    S      = sum_n Wf.reshape(C, N, C)[:, n, :]   (C, C)
    out    = w_row @ S.T + bf                     (B, C)

Sharding: each of the 8 cores owns 8 output channels c' (8 contiguous rows
of Wf), reduces them to S^T[:, c'_range] on-device, and computes its disjoint
slice out^T[c'_range, :].  The tiny upstream GEMMs are replicated per core.

Optimizations over the previous (2127 ns) version:
  - Wf ships as fp8e3 (e3m4) instead of fp16 -- 256 KiB/core instead of 512.
    The quantizer uses error diffusion along n (the axis the device sums
    over): the residual of each cast is carried into the next element, so the
    *sum* of the shipped fp8 values matches the fp32 sum to within one
    quantum.  End-to-end rel err ~6e-4 (vs 1.1e-2 for naive fp8).
  - The n-reduction moves from DVE tensor_reduce (1x mode, ~2.2 us for 2048
    cols) to TensorE: matmuls against a constant block-identity mask
    contract 128 partition rows at a time at 2.4 GHz.  Only a short
    [*, 8*NI] -> [*, 8] tail reduce stays on DVE.
  - Optional 2-way column tiling (KV_MODE=coltile, NOT default): two concurrent
    matmul streams on array column groups 0-63 / 64-127 halve the PE
    streaming time for the Wf reduction.  The two partition-halves of S^T
    are summed for free inside the final matmul by duplicating w_row^T rows
    via a 0-stride lhs access pattern.
  - elu recombined as (relu(x) - 1) + exp(-relu(-x)) with one fused DVE
    scalar_tensor_tensor; final out = (o_p * sWf) + bf is one fused DVE
    tensor_scalar with both scalars riding as per-partition data (so the
    compiled program has no input-derived immediates).

Per-core DMA: wf 256 KiB fp8 + small pack ~201 KiB fp16 = ~457 KiB.
Shipping config (measured best, HW differential ~1.0-1.1 us/iter vs 2.1 us
baseline): MODE=plain FD=128, ELU=dve (exp on ACT, everything else on DVE),
FINAL=dve, separate small/wf DMAs.  Rejected by measurement: coltile (x2
slower -- tile_position forces a serialized LDWEIGHTS before every matmul),
FD=256 (+370 ns), fused single DMA (x4 slower -- kills cross-rep
pipelining), ELU=act (ACT's 3x400 ns serial chain binds).
"""

import os
import numpy as np
import ml_dtypes

import concourse.bass as bass
import concourse.bacc as bacc
import concourse.mybir as mybir
import concourse.tile as tile
from concourse.bass_utils import run_bass_kernel_spmd

N_CORES = 8
B, S_SEQ, F = 32, 12, 128
K, H, C, N = 8, 64, 64, 512
ROWS = C // N_CORES          # output channels per core
F32 = mybir.dt.float32
F16 = mybir.dt.float16
F8E3 = mybir.dt.float8e3
AF = mybir.ActivationFunctionType
ALU = mybir.AluOpType

MODE = os.environ.get("KV_MODE", "plain")     # coltile | plain
FD = int(os.environ.get("KV_FD", "128"))      # rhs cols per wf matmul (plain)
ELU = os.environ.get("KV_ELU", "dve")         # act | mid | dve
FINAL = os.environ.get("KV_FINAL", "dve")     # act | dve
FUSE = os.environ.get("KV_FUSE_DMA", "0") == "1"   # single input DMA per rep
WF_DMAS = int(os.environ.get("KV_WF_DMAS", "2"))   # 1 | 2 wf transfers
if MODE == "coltile":
    NI = 8
    NKCHUNK = 16
else:
    NI = FD // 8                              # psum ni-width per c'
    NKCHUNK = 2048 // FD                      # accumulation steps per stream

# small-pack column layout (fp16): zt | wht | wot | aux-bits
ZT0 = 0
WHT0 = ZT0 + B
WOT0 = WHT0 + K * H
AUXBITS0 = WOT0 + 4 * C      # even => fp32 bitcast view is 4B aligned
BFT_COL = 0                  # aux fp32 col 0: bias (rows 0..ROWS)
SWF_COL = 1                  # aux fp32 col 1: Wf scale (rows 0..ROWS)
AUX_COLS = 2
SMALL_COLS = AUXBITS0 + 2 * AUX_COLS


def _emit_consts(nc, cpool, t):
    """Hoisted once per program: the block-identity reduction mask."""
    mask_s = cpool.tile([128, C], F8E3, tag="mask", name="mask")
    nc.sync.dma_start(mask_s[:], t["cst"][:])
    return mask_s


def _emit_body(nc, pool, wfpool, psum, t, mask_s, tc=None):
    """One full per-core computation; `t` maps dram tensor names to handles."""
    if FUSE:
        all_s = wfpool.tile([128, 2048 + 2 * SMALL_COLS], F8E3, tag="all")
        nc.sync.dma_start(all_s[:], t["wfall"][:])
        wf_view = all_s[:, 0:2048]
        wf_slices = [wf_view[:, 1024 * ci:1024 * (ci + 1)] for ci in range(2)]
        small_s = all_s[:, 2048:2048 + 2 * SMALL_COLS].bitcast(F16)
    elif WF_DMAS == 1:
        small_t = pool.tile([128, SMALL_COLS], F16, tag="small")
        nc.sync.dma_start(small_t[:], t["small"][:])
        small_s = small_t[:]
        wf_tile = wfpool.tile([128, 2048], F8E3, tag="wfchunk", name="wfchunk")
        nc.sync.dma_start(wf_tile[:], t["wf"][:])
        wf_slices = [wf_tile[:, 0:1024], wf_tile[:, 1024:2048]]
    else:
        small_t = pool.tile([128, SMALL_COLS], F16, tag="small")
        nc.sync.dma_start(small_t[:], t["small"][:])
        small_s = small_t[:]
        wf_tiles = [wfpool.tile([128, 1024], F8E3, tag=f"wfchunk{ci}",
                                name=f"wfchunk{ci}")
                    for ci in range(2)]
        for ci in range(2):
            nc.sync.dma_start(wf_tiles[ci][:],
                              t["wf"][:, 1024 * ci:1024 * (ci + 1)])
        wf_slices = [wf_tiles[ci][:] for ci in range(2)]

    zt_s = small_s[:, ZT0:ZT0 + B]
    wht_s = small_s[:, WHT0:WHT0 + K * H]
    wot_s = small_s[:, WOT0:WOT0 + 4 * C]
    aux_v = small_s[:, AUXBITS0:AUXBITS0 + 2 * AUX_COLS].bitcast(F32)
    bft_s = aux_v[0:ROWS, BFT_COL:BFT_COL + 1]
    swf_s = aux_v[0:ROWS, SWF_COL:SWF_COL + 1]

    # ---- u-pre = W_heads^T z  (4 chunks of 128 kh each) ---------------
    wh_p = psum.tile([128, 4 * B], F32, tag="whp")
    for j in range(4):
        nc.tensor.matmul(
            wh_p[:, B * j:B * (j + 1)],
            wht_s[:, 128 * j:128 * (j + 1)],
            zt_s,
            start=True, stop=True,
        )

    # ---- S^T from the Wf shard: PE mask-matmul reduction --------------
    # wf col layout (plain):   j = k*128 + c'l*16 + ni   (h = k*16 + ni)
    # wf col layout (coltile): j = k*128 + T*64 + c'l*8 + ni
    #                          (h = T*128 + k*8 + ni), T = array col group
    if MODE == "coltile":
        st_p = psum.tile([128, C], F32, tag="stp")
        for k in range(NKCHUNK):
            ci, off = k // 8, (k % 8) * 128
            for T in range(2):
                nc.tensor.matmul(
                    st_p[64 * T:64 * (T + 1), :],
                    mask_s[:],
                    wf_slices[ci][:, off + 64 * T:off + 64 * (T + 1)],
                    start=(k == 0), stop=(k == NKCHUNK - 1),
                )
        st_rows = 128
    else:
        kperchunk = NKCHUNK // 2
        st_p = psum.tile([C, 8 * NI], F32, tag="stp")
        for k in range(NKCHUNK):
            ci, off = k // kperchunk, (k % kperchunk) * FD
            nc.tensor.matmul(
                st_p[:],
                mask_s[:],
                wf_slices[ci][:, off:off + FD],
                start=(k == 0), stop=(k == NKCHUNK - 1),
            )
        st_rows = C

    # ---- elu: u = (relu(x) - 1) + exp(min(x, 0)) ----------------------
    u_s = pool.tile([128, 4 * B], F16, tag="u")
    e1_s = pool.tile([128, 4 * B], F16, tag="e1")
    if ELU == "dve":
        m_s = pool.tile([128, 4 * B], F16, tag="m")
        nc.vector.tensor_scalar_min(m_s[:], wh_p[:], 0.0)
        nc.scalar.activation(e1_s[:], m_s[:], AF.Exp)
        r1_s = pool.tile([128, 4 * B], F16, tag="r1")
        nc.vector.tensor_scalar(
            r1_s[:], wh_p[:], 0.0, -1.0, op0=ALU.max, op1=ALU.add)
        nc.vector.tensor_add(u_s[:], r1_s[:], e1_s[:])
    elif ELU == "mid":
        rneg_s = pool.tile([128, 4 * B], F16, tag="rneg")
        nc.scalar.activation(rneg_s[:], wh_p[:], AF.Relu, scale=-1.0)
        nc.scalar.activation(e1_s[:], rneg_s[:], AF.Exp, scale=-1.0)
        r1_s = pool.tile([128, 4 * B], F16, tag="r1")
        nc.vector.tensor_scalar(
            r1_s[:], wh_p[:], 0.0, -1.0, op0=ALU.max, op1=ALU.add)
        nc.vector.tensor_add(u_s[:], r1_s[:], e1_s[:])
    else:
        rneg_s = pool.tile([128, 4 * B], F16, tag="rneg")
        nc.scalar.activation(rneg_s[:], wh_p[:], AF.Relu, scale=-1.0)
        nc.scalar.activation(e1_s[:], rneg_s[:], AF.Exp, scale=-1.0)
        r_s = pool.tile([128, 4 * B], F16, tag="r")
        nc.scalar.activation(r_s[:], wh_p[:], AF.Relu)
        nc.vector.scalar_tensor_tensor(
            u_s[:], r_s[:], -1.0, e1_s[:], op0=ALU.add, op1=ALU.add)

    # ---- w_row^T = W_out^T u ------------------------------------------
    # coltile: also materialize a copy of w_row^T on partitions 64..127
    # (array col groups 2-3, concurrent with groups 0-1) so the final
    # matmul's 128-partition contraction sums the two S^T halves for free.
    wr_p = psum.tile([st_rows, B], F32, tag="wrp")
    halves = 2 if MODE == "coltile" else 1
    for j in range(4):
        wot_j = wot_s[:, C * j:C * (j + 1)]
        for hf in range(halves):
            nc.tensor.matmul(
                wr_p[64 * hf:64 * hf + 64, :] if halves == 2 else wr_p[:],
                wot_j, u_s[:, B * j:B * (j + 1)],
                start=(j == 0), stop=(j == 3),
            )
    wr_s = pool.tile([st_rows, B], F32, tag="wrs")
    nc.vector.tensor_copy(wr_s[:], wr_p[:])

    # ---- tail reduce over ni: S^T slice -------------------------------
    st_s = pool.tile([st_rows, ROWS], F32, tag="sts")
    nc.vector.tensor_reduce(
        st_s[:],
        st_p[:].rearrange("p (c n) -> p c n", n=NI),
        axis=mybir.AxisListType.X,
        op=ALU.add,
    )

    # ---- out^T[c' slice] = sWf * (S^T.T w_row^T) + bf -----------------
    o_p = psum.tile([ROWS, B], F32, tag="op")
    nc.tensor.matmul(o_p[:], st_s[:], wr_s[:], start=True, stop=True)
    o_s = pool.tile([ROWS, B], F32, tag="os")
    if FINAL == "act":
        nc.scalar.activation(o_s[:], o_p[:], AF.Identity,
                             bias=bft_s, scale=swf_s)
    else:
        nc.vector.tensor_scalar(
            o_s[:], o_p[:], swf_s, bft_s, op0=ALU.mult, op1=ALU.add)
    nc.sync.dma_start(t["out"][:], o_s[:])


def _build_nc(reps=1, loop_iters=None):
    nc = bacc.Bacc("TRN2", target_bir_lowering=False, debug=False,
                   num_devices=N_CORES)

    if FUSE:
        t = {
            "wfall": nc.dram_tensor("wfall", [128, 2048 + 2 * SMALL_COLS],
                                    F8E3, kind="ExternalInput"),
            "cst": nc.dram_tensor("cst", [128, C], F8E3, kind="ExternalInput"),
            "out": nc.dram_tensor("out", [ROWS, B], F32, kind="ExternalOutput"),
        }
    else:
        t = {
            "wf": nc.dram_tensor("wf", [128, 2048], F8E3, kind="ExternalInput"),
            "small": nc.dram_tensor("small", [128, SMALL_COLS], F16,
                                    kind="ExternalInput"),
            "cst": nc.dram_tensor("cst", [128, C], F8E3, kind="ExternalInput"),
            "out": nc.dram_tensor("out", [ROWS, B], F32, kind="ExternalOutput"),
        }

    with tile.TileContext(nc) as tc:
        with (
            tc.tile_pool(name="cpool", bufs=1) as cpool,
            tc.tile_pool(name="pool", bufs=int(os.environ.get("KV_POOL_BUFS", "3"))) as pool,
            tc.tile_pool(name="wfpool", bufs=int(os.environ.get("KV_WF_BUFS", "2"))) as wfpool,
            tc.tile_pool(name="psum", bufs=2, space=bass.MemorySpace.PSUM) as psum,
        ):
            mask_s = _emit_consts(nc, cpool, t)
            if loop_iters:
                tc.For_i_unrolled(
                    0, loop_iters, 1,
                    lambda iv: _emit_body(nc, pool, wfpool, psum, t, mask_s, tc),
                    max_unroll=int(os.environ.get("KV_UNROLL", "64")),
                )
            else:
                for _rep in range(reps):
                    _emit_body(nc, pool, wfpool, psum, t, mask_s, tc)

    nc.compile()
    return nc


_NC_CACHE = None
_last_in_maps = None


def _quant_wf_feedback(Wf):
    """fp8e3 quantization of Wf with error diffusion along n (the summed
    axis): sum_n q[:, n, :] == sum_n Wf[:, n, :] to within one quantum."""
    m = float(np.abs(Wf).max())
    swf = float(2.0 ** np.ceil(np.log2(m / 7.75))) if m > 0 else 1.0
    W = (Wf / swf).reshape(C, N, C).astype(np.float32)
    q = np.empty((C, N, C), dtype=ml_dtypes.float8_e3m4)
    carry = np.zeros((C, C), np.float32)
    for n in range(N):
        tgt = W[:, n, :] + carry
        qn = tgt.astype(ml_dtypes.float8_e3m4)
        carry = tgt - qn.astype(np.float32)
        q[:, n, :] = qn
    return q.reshape(C, N * C), swf


def _make_in_maps(x, W_heads, W_out, Wf, bf):
    x = np.ascontiguousarray(np.asarray(x, np.float32))
    W_heads = np.ascontiguousarray(np.asarray(W_heads, np.float32))
    W_out = np.ascontiguousarray(np.asarray(W_out, np.float32))
    Wf = np.ascontiguousarray(np.asarray(Wf, np.float32))
    bf = np.ascontiguousarray(np.asarray(bf, np.float32))

    small = np.zeros((128, SMALL_COLS), np.float16)
    small[:, ZT0:ZT0 + B] = x[:, -1, :].T                          # (128, 32)
    small[:, WHT0:WHT0 + K * H] = \
        W_heads.transpose(1, 0, 2).reshape(F, K * H)               # (128, 512)
    small[:, WOT0:WOT0 + 4 * C] = \
        W_out.reshape(4, 128, C).transpose(1, 0, 2).reshape(128, 4 * C)

    qWf, swf = _quant_wf_feedback(Wf)                              # (64, 32768)

    # constant block-identity mask: mask[p, c] = (p % 64 == c)
    maskh = np.zeros((128, C), dtype=ml_dtypes.float8_e3m4)
    pp = np.arange(128)
    maskh[pp, pp % C] = ml_dtypes.float8_e3m4(1.0)

    in_maps = []
    for core in range(N_CORES):
        shard = qWf[ROWS * core:ROWS * (core + 1)]                 # (8, 32768)
        sh = shard.reshape(ROWS, 256, 128)                         # [c'l, h, p]
        if MODE == "coltile":
            # h = T*128 + k*8 + ni ; col j = k*128 + T*64 + c'l*8 + ni
            g = sh.reshape(ROWS, 2, 16, 8, 128)                    # [c'l,T,k,ni,p]
            wf_host = np.ascontiguousarray(
                g.transpose(4, 2, 1, 0, 3)).reshape(128, 2048)     # [p,k,T,c'l,ni]
        else:
            # h = k*NI + ni ; col j = k*FD + c'l*NI + ni
            g = sh.reshape(ROWS, NKCHUNK, NI, 128)                 # [c'l,k,ni,p]
            wf_host = np.ascontiguousarray(
                g.transpose(3, 1, 0, 2)).reshape(128, 2048)        # [p,k,c'l,ni]

        aux = np.zeros((128, AUX_COLS), np.float32)
        aux[0:ROWS, BFT_COL] = bf[ROWS * core:ROWS * (core + 1)]
        aux[0:ROWS, SWF_COL] = swf
        small_c = small.copy()
        small_c[:, AUXBITS0:AUXBITS0 + 2 * AUX_COLS] = aux.view(np.float16)
        if FUSE:
            wfall = np.concatenate(
                [wf_host.view(np.uint8), small_c.view(np.uint8)], axis=1
            ).view(ml_dtypes.float8_e3m4)
            in_maps.append({"wfall": wfall, "cst": maskh})
        else:
            in_maps.append({"wf": wf_host, "small": small_c, "cst": maskh})
    return in_maps


def kernel(x, W_heads, a1_heads, a2_heads, W_out, a1_out, a2_out, Wf, bf):
    global _NC_CACHE
    if _NC_CACHE is None:
        _NC_CACHE = _build_nc()
    nc = _NC_CACHE

    in_maps = _make_in_maps(x, W_heads, W_out, Wf, bf)
    global _last_in_maps
    _last_in_maps = in_maps
    res = run_bass_kernel_spmd(nc, in_maps, list(range(N_CORES)))
    outT = np.concatenate([res.results[i]["out"] for i in range(N_CORES)], axis=0)
    return np.ascontiguousarray(outT.T)                            # (32, 64)


# revision 24
# speedup vs baseline: 2.3670x; 1.4597x over previous
"""Trainium2 Bass kernel for nn_GATTrafficPredictionModel.

Mathematical collapse exploited (holds for every input by construction of the
model, not by luck of the data):
  - h = broadcast(x[:, -1, :]) makes all N=512 node features identical per
    sample, and the adjacency is dense all-ones.
  - GAT attention scores e[i,j] = leakyrelu(s_src[i] + s_dst[j]) are therefore
    constant over (i, j), so softmax over neighbors is exactly uniform (1/512,
    exact in fp32), and the attention-weighted sum of identical rows
    reproduces the row itself.  Both GAT layers collapse to per-sample linear
    maps; a1/a2 attention vectors drop out entirely.

Collapsed computation (B=32, F=128, K=8, H=64, C=64, N=512):
    z      = x[:, -1, :]                          (B, F)
    u      = elu(z @ W_heads)  flattened heads    (B, K*H)
    w_row  = u @ W_out                            (B, C)
    S      = sum_n Wf.reshape(C, N, C)[:, n, :]   (C, C)
    out    = w_row @ S.T + bf                     (B, C)

Sharding: each of the 8 cores owns 8 output channels c' (8 contiguous rows
of Wf), reduces them to S^T[:, c'_range] on-device, and computes its disjoint
slice out^T[c'_range, :].  The tiny upstream GEMMs are replicated per core.

Optimizations over the previous (2127 ns) version:
  - Wf ships as fp8e3 (e3m4) instead of fp16 -- 256 KiB/core instead of 512.
    The quantizer uses error diffusion along n (the axis the device sums
    over): the residual of each cast is carried into the next element, so the
    *sum* of the shipped fp8 values matches the fp32 sum to within one
    quantum.  End-to-end rel err ~6e-4 (vs 1.1e-2 for naive fp8).
  - The n-reduction moves from DVE tensor_reduce (1x mode, ~2.2 us for 2048
    cols) to TensorE: matmuls against a constant block-identity mask
    contract 128 partition rows at a time at 2.4 GHz.  Only a short
    [*, 8*NI] -> [*, 8] tail reduce stays on DVE.
  - Optional 2-way column tiling (KV_MODE=coltile, NOT default): two concurrent
    matmul streams on array column groups 0-63 / 64-127 halve the PE
    streaming time for the Wf reduction.  The two partition-halves of S^T
    are summed for free inside the final matmul by duplicating w_row^T rows
    via a 0-stride lhs access pattern.
  - elu recombined as (relu(x) - 1) + exp(-relu(-x)) with one fused DVE
    scalar_tensor_tensor; final out = (o_p * sWf) + bf is one fused DVE
    tensor_scalar with both scalars riding as per-partition data (so the
    compiled program has no input-derived immediates).

Per-core DMA: wf 256 KiB fp8 + small pack ~201 KiB fp16 = ~457 KiB.
Shipping config (measured best, HW differential ~1.0-1.4 us/iter vs 2.1 us
baseline): MODE=plain FD=128, ELU=dve (exp on ACT, elu recombine on DVE),
FINAL=act (out scale+bias on the otherwise-idle ACT via scale/bias APs),
WF_DMAS=1 (one 2 KiB/partition wf transfer), separate small-pack DMA.
Rejected by measurement: coltile (x2 slower -- tile_position forces a
serialized LDWEIGHTS before every matmul), FD=256 (+370 ns), fully fused
single DMA (x4 slower -- kills cross-rep pipelining), ELU=act (ACT's
3x400 ns serial chain binds), hardware loops (bodies serialize, 3.7 us/it).
"""

import os
import numpy as np
import ml_dtypes

import concourse.bass as bass
import concourse.bacc as bacc
import concourse.mybir as mybir
import concourse.tile as tile
from concourse.bass_utils import run_bass_kernel_spmd

N_CORES = 8
B, S_SEQ, F = 32, 12, 128
K, H, C, N = 8, 64, 64, 512
ROWS = C // N_CORES          # output channels per core
F32 = mybir.dt.float32
F16 = mybir.dt.float16
F8E3 = mybir.dt.float8e3
AF = mybir.ActivationFunctionType
ALU = mybir.AluOpType

MODE = os.environ.get("KV_MODE", "plain")     # coltile | plain
FD = int(os.environ.get("KV_FD", "128"))      # rhs cols per wf matmul (plain)
ELU = os.environ.get("KV_ELU", "dve")         # act | mid | dve
FINAL = os.environ.get("KV_FINAL", "act")     # act | dve
FUSE = os.environ.get("KV_FUSE_DMA", "0") == "1"   # single input DMA per rep
WF_DMAS = int(os.environ.get("KV_WF_DMAS", "1"))   # 1 | 2 wf transfers
if MODE == "coltile":
    NI = 8
    NKCHUNK = 16
else:
    NI = FD // 8                              # psum ni-width per c'
    NKCHUNK = 2048 // FD                      # accumulation steps per stream

# small-pack column layout (fp16): zt | wht | wot | aux-bits
ZT0 = 0
WHT0 = ZT0 + B
WOT0 = WHT0 + K * H
AUXBITS0 = WOT0 + 4 * C      # even => fp32 bitcast view is 4B aligned
BFT_COL = 0                  # aux fp32 col 0: bias (rows 0..ROWS)
SWF_COL = 1                  # aux fp32 col 1: Wf scale (rows 0..ROWS)
AUX_COLS = 2
SMALL_COLS = AUXBITS0 + 2 * AUX_COLS


def _emit_consts(nc, cpool, t):
    """Hoisted once per program: the block-identity reduction mask."""
    mask_s = cpool.tile([128, C], F8E3, tag="mask", name="mask")
    nc.sync.dma_start(mask_s[:], t["cst"][:])
    return mask_s


def _emit_body(nc, pool, wfpool, psum, t, mask_s, tc=None):
    """One full per-core computation; `t` maps dram tensor names to handles."""
    if FUSE:
        all_s = wfpool.tile([128, 2048 + 2 * SMALL_COLS], F8E3, tag="all")
        nc.sync.dma_start(all_s[:], t["wfall"][:])
        wf_view = all_s[:, 0:2048]
        wf_slices = [wf_view[:, 1024 * ci:1024 * (ci + 1)] for ci in range(2)]
        small_s = all_s[:, 2048:2048 + 2 * SMALL_COLS].bitcast(F16)
    elif WF_DMAS == 1:
        small_t = pool.tile([128, SMALL_COLS], F16, tag="small")
        nc.sync.dma_start(small_t[:], t["small"][:])
        small_s = small_t[:]
        wf_tile = wfpool.tile([128, 2048], F8E3, tag="wfchunk", name="wfchunk")
        nc.sync.dma_start(wf_tile[:], t["wf"][:])
        wf_slices = [wf_tile[:, 0:1024], wf_tile[:, 1024:2048]]
    else:
        small_t = pool.tile([128, SMALL_COLS], F16, tag="small")
        nc.sync.dma_start(small_t[:], t["small"][:])
        small_s = small_t[:]
        wf_tiles = [wfpool.tile([128, 1024], F8E3, tag=f"wfchunk{ci}",
                                name=f"wfchunk{ci}")
                    for ci in range(2)]
        for ci in range(2):
            nc.sync.dma_start(wf_tiles[ci][:],
                              t["wf"][:, 1024 * ci:1024 * (ci + 1)])
        wf_slices = [wf_tiles[ci][:] for ci in range(2)]

    zt_s = small_s[:, ZT0:ZT0 + B]
    wht_s = small_s[:, WHT0:WHT0 + K * H]
    wot_s = small_s[:, WOT0:WOT0 + 4 * C]
    aux_v = small_s[:, AUXBITS0:AUXBITS0 + 2 * AUX_COLS].bitcast(F32)
    bft_s = aux_v[0:ROWS, BFT_COL:BFT_COL + 1]
    swf_s = aux_v[0:ROWS, SWF_COL:SWF_COL + 1]

    # ---- u-pre = W_heads^T z  (4 chunks of 128 kh each) ---------------
    wh_p = psum.tile([128, 4 * B], F32, tag="whp")
    for j in range(4):
        nc.tensor.matmul(
            wh_p[:, B * j:B * (j + 1)],
            wht_s[:, 128 * j:128 * (j + 1)],
            zt_s,
            start=True, stop=True,
        )

    # ---- S^T from the Wf shard: PE mask-matmul reduction --------------
    # wf col layout (plain):   j = k*128 + c'l*16 + ni   (h = k*16 + ni)
    # wf col layout (coltile): j = k*128 + T*64 + c'l*8 + ni
    #                          (h = T*128 + k*8 + ni), T = array col group
    if MODE == "coltile":
        st_p = psum.tile([128, C], F32, tag="stp")
        for k in range(NKCHUNK):
            ci, off = k // 8, (k % 8) * 128
            for T in range(2):
                nc.tensor.matmul(
                    st_p[64 * T:64 * (T + 1), :],
                    mask_s[:],
                    wf_slices[ci][:, off + 64 * T:off + 64 * (T + 1)],
                    start=(k == 0), stop=(k == NKCHUNK - 1),
                )
        st_rows = 128
    else:
        kperchunk = NKCHUNK // 2
        st_p = psum.tile([C, 8 * NI], F32, tag="stp")
        for k in range(NKCHUNK):
            ci, off = k // kperchunk, (k % kperchunk) * FD
            nc.tensor.matmul(
                st_p[:],
                mask_s[:],
                wf_slices[ci][:, off:off + FD],
                start=(k == 0), stop=(k == NKCHUNK - 1),
            )
        st_rows = C

    # ---- elu: u = (relu(x) - 1) + exp(min(x, 0)) ----------------------
    u_s = pool.tile([128, 4 * B], F16, tag="u")
    e1_s = pool.tile([128, 4 * B], F16, tag="e1")
    if ELU == "dve":
        m_s = pool.tile([128, 4 * B], F16, tag="m")
        nc.vector.tensor_scalar_min(m_s[:], wh_p[:], 0.0)
        nc.scalar.activation(e1_s[:], m_s[:], AF.Exp)
        r1_s = pool.tile([128, 4 * B], F16, tag="r1")
        nc.vector.tensor_scalar(
            r1_s[:], wh_p[:], 0.0, -1.0, op0=ALU.max, op1=ALU.add)
        nc.vector.tensor_add(u_s[:], r1_s[:], e1_s[:])
    elif ELU == "mid":
        rneg_s = pool.tile([128, 4 * B], F16, tag="rneg")
        nc.scalar.activation(rneg_s[:], wh_p[:], AF.Relu, scale=-1.0)
        nc.scalar.activation(e1_s[:], rneg_s[:], AF.Exp, scale=-1.0)
        r1_s = pool.tile([128, 4 * B], F16, tag="r1")
        nc.vector.tensor_scalar(
            r1_s[:], wh_p[:], 0.0, -1.0, op0=ALU.max, op1=ALU.add)
        nc.vector.tensor_add(u_s[:], r1_s[:], e1_s[:])
    else:
        rneg_s = pool.tile([128, 4 * B], F16, tag="rneg")
        nc.scalar.activation(rneg_s[:], wh_p[:], AF.Relu, scale=-1.0)
        nc.scalar.activation(e1_s[:], rneg_s[:], AF.Exp, scale=-1.0)
        r_s = pool.tile([128, 4 * B], F16, tag="r")
        nc.scalar.activation(r_s[:], wh_p[:], AF.Relu)
        nc.vector.scalar_tensor_tensor(
            u_s[:], r_s[:], -1.0, e1_s[:], op0=ALU.add, op1=ALU.add)

    # ---- w_row^T = W_out^T u ------------------------------------------
    # coltile: also materialize a copy of w_row^T on partitions 64..127
    # (array col groups 2-3, concurrent with groups 0-1) so the final
    # matmul's 128-partition contraction sums the two S^T halves for free.
    wr_p = psum.tile([st_rows, B], F32, tag="wrp")
    halves = 2 if MODE == "coltile" else 1
    for j in range(4):
        wot_j = wot_s[:, C * j:C * (j + 1)]
        for hf in range(halves):
            nc.tensor.matmul(
                wr_p[64 * hf:64 * hf + 64, :] if halves == 2 else wr_p[:],
                wot_j, u_s[:, B * j:B * (j + 1)],
                start=(j == 0), stop=(j == 3),
            )
    wr_s = pool.tile([st_rows, B], F32, tag="wrs")
    nc.vector.tensor_copy(wr_s[:], wr_p[:])

    # ---- tail reduce over ni: S^T slice -------------------------------
    st_s = pool.tile([st_rows, ROWS], F32, tag="sts")
    nc.vector.tensor_reduce(
        st_s[:],
        st_p[:].rearrange("p (c n) -> p c n", n=NI),
        axis=mybir.AxisListType.X,
        op=ALU.add,
    )

    # ---- out^T[c' slice] = sWf * (S^T.T w_row^T) + bf -----------------
    o_p = psum.tile([ROWS, B], F32, tag="op")
    nc.tensor.matmul(o_p[:], st_s[:], wr_s[:], start=True, stop=True)
    o_s = pool.tile([ROWS, B], F32, tag="os")
    if FINAL == "act":
        nc.scalar.activation(o_s[:], o_p[:], AF.Identity,
                             bias=bft_s, scale=swf_s)
    else:
        nc.vector.tensor_scalar(
            o_s[:], o_p[:], swf_s, bft_s, op0=ALU.mult, op1=ALU.add)
    nc.sync.dma_start(t["out"][:], o_s[:])


def _build_nc(reps=1, loop_iters=None):
    nc = bacc.Bacc("TRN2", target_bir_lowering=False, debug=False,
                   num_devices=N_CORES)

    if FUSE:
        t = {
            "wfall": nc.dram_tensor("wfall", [128, 2048 + 2 * SMALL_COLS],
                                    F8E3, kind="ExternalInput"),
            "cst": nc.dram_tensor("cst", [128, C], F8E3, kind="ExternalInput"),
            "out": nc.dram_tensor("out", [ROWS, B], F32, kind="ExternalOutput"),
        }
    else:
        t = {
            "wf": nc.dram_tensor("wf", [128, 2048], F8E3, kind="ExternalInput"),
            "small": nc.dram_tensor("small", [128, SMALL_COLS], F16,
                                    kind="ExternalInput"),
            "cst": nc.dram_tensor("cst", [128, C], F8E3, kind="ExternalInput"),
            "out": nc.dram_tensor("out", [ROWS, B], F32, kind="ExternalOutput"),
        }

    with tile.TileContext(nc) as tc:
        with (
            tc.tile_pool(name="cpool", bufs=1) as cpool,
            tc.tile_pool(name="pool", bufs=int(os.environ.get("KV_POOL_BUFS", "3"))) as pool,
            tc.tile_pool(name="wfpool", bufs=int(os.environ.get("KV_WF_BUFS", "2"))) as wfpool,
            tc.tile_pool(name="psum", bufs=2, space=bass.MemorySpace.PSUM) as psum,
        ):
            mask_s = _emit_consts(nc, cpool, t)
            if loop_iters:
                tc.For_i_unrolled(
                    0, loop_iters, 1,
                    lambda iv: _emit_body(nc, pool, wfpool, psum, t, mask_s, tc),
                    max_unroll=int(os.environ.get("KV_UNROLL", "64")),
                )
            else:
                for _rep in range(reps):
                    _emit_body(nc, pool, wfpool, psum, t, mask_s, tc)

    nc.compile()
    return nc


_NC_CACHE = None
_last_in_maps = None


def _quant_wf_feedback(Wf):
    """fp8e3 quantization of Wf with error diffusion along n (the summed
    axis): sum_n q[:, n, :] == sum_n Wf[:, n, :] to within one quantum."""
    m = float(np.abs(Wf).max())
    swf = float(2.0 ** np.ceil(np.log2(m / 7.75))) if m > 0 else 1.0
    W = (Wf / swf).reshape(C, N, C).astype(np.float32)
    q = np.empty((C, N, C), dtype=ml_dtypes.float8_e3m4)
    carry = np.zeros((C, C), np.float32)
    for n in range(N):
        tgt = W[:, n, :] + carry
        qn = tgt.astype(ml_dtypes.float8_e3m4)
        carry = tgt - qn.astype(np.float32)
        q[:, n, :] = qn
    return q.reshape(C, N * C), swf


def _make_in_maps(x, W_heads, W_out, Wf, bf):
    x = np.ascontiguousarray(np.asarray(x, np.float32))
    W_heads = np.ascontiguousarray(np.asarray(W_heads, np.float32))
    W_out = np.ascontiguousarray(np.asarray(W_out, np.float32))
    Wf = np.ascontiguousarray(np.asarray(Wf, np.float32))
    bf = np.ascontiguousarray(np.asarray(bf, np.float32))

    small = np.zeros((128, SMALL_COLS), np.float16)
    small[:, ZT0:ZT0 + B] = x[:, -1, :].T                          # (128, 32)
    small[:, WHT0:WHT0 + K * H] = \
        W_heads.transpose(1, 0, 2).reshape(F, K * H)               # (128, 512)
    small[:, WOT0:WOT0 + 4 * C] = \
        W_out.reshape(4, 128, C).transpose(1, 0, 2).reshape(128, 4 * C)

    qWf, swf = _quant_wf_feedback(Wf)                              # (64, 32768)

    # constant block-identity mask: mask[p, c] = (p % 64 == c)
    maskh = np.zeros((128, C), dtype=ml_dtypes.float8_e3m4)
    pp = np.arange(128)
    maskh[pp, pp % C] = ml_dtypes.float8_e3m4(1.0)

    in_maps = []
    for core in range(N_CORES):
        shard = qWf[ROWS * core:ROWS * (core + 1)]                 # (8, 32768)
        sh = shard.reshape(ROWS, 256, 128)                         # [c'l, h, p]
        if MODE == "coltile":
            # h = T*128 + k*8 + ni ; col j = k*128 + T*64 + c'l*8 + ni
            g = sh.reshape(ROWS, 2, 16, 8, 128)                    # [c'l,T,k,ni,p]
            wf_host = np.ascontiguousarray(
                g.transpose(4, 2, 1, 0, 3)).reshape(128, 2048)     # [p,k,T,c'l,ni]
        else:
            # h = k*NI + ni ; col j = k*FD + c'l*NI + ni
            g = sh.reshape(ROWS, NKCHUNK, NI, 128)                 # [c'l,k,ni,p]
            wf_host = np.ascontiguousarray(
                g.transpose(3, 1, 0, 2)).reshape(128, 2048)        # [p,k,c'l,ni]

        aux = np.zeros((128, AUX_COLS), np.float32)
        aux[0:ROWS, BFT_COL] = bf[ROWS * core:ROWS * (core + 1)]
        aux[0:ROWS, SWF_COL] = swf
        small_c = small.copy()
        small_c[:, AUXBITS0:AUXBITS0 + 2 * AUX_COLS] = aux.view(np.float16)
        if FUSE:
            wfall = np.concatenate(
                [wf_host.view(np.uint8), small_c.view(np.uint8)], axis=1
            ).view(ml_dtypes.float8_e3m4)
            in_maps.append({"wfall": wfall, "cst": maskh})
        else:
            in_maps.append({"wf": wf_host, "small": small_c, "cst": maskh})
    return in_maps


def kernel(x, W_heads, a1_heads, a2_heads, W_out, a1_out, a2_out, Wf, bf):
    global _NC_CACHE
    if _NC_CACHE is None:
        _NC_CACHE = _build_nc()
    nc = _NC_CACHE

    in_maps = _make_in_maps(x, W_heads, W_out, Wf, bf)
    global _last_in_maps
    _last_in_maps = in_maps
    res = run_bass_kernel_spmd(nc, in_maps, list(range(N_CORES)))
    outT = np.concatenate([res.results[i]["out"] for i in range(N_CORES)], axis=0)
    return np.ascontiguousarray(outT.T)                            # (32, 64)
